# revision 33
# baseline (speedup 1.0000x reference)
"""Trainium2 Bass kernel v4 for nn_Attention_4810363372413.

GQA attention: B=2, S=2048, E=2048, HQ=32, HK=8, D=64, RoPE, zero mask,
no 1/sqrt(d) scaling. 8 cores: core c owns kv-head c, q-heads 4c..4c+3.
Each core computes a bf16 partial over its 4 heads; host sums partials.

Key design vs v3:
- Scores matmuls in bf16 (qcomb/kq_t bf16): bf16 moving operand streams
  2 cols/cycle on HW vs fp32's 1 -> halves the scores PE time.
  Measured rel err ~1.2e-2 (gate 2e-2).
- 1/z broadcast moved off PE (was a K=1 outer-product matmul) onto the
  idle GpSimd engine via partition_broadcast (attn ucode library).
- All oproj PSUM->SBUF copies on DVE (ACT is the critical engine: exp
  is (N+352)/1.2 ns per activation and there are 256 of them).
- Output assembled into a [128, 2048] staging tile, one 512 KB DMA per
  token chunk (4 KB per-partition lines) instead of 4x 1 KB-line DMAs.
- oproj(b-1) + qkv(b+1) matmuls interleaved into attention(b) to keep
  PE dense while ACT paces the kt loop.
"""

import os
import sys

sys.path.insert(0, "/opt/trn_rl_repo")

import numpy as np

B, S, E = 2, 2048, 2048
HQ, HK, D = 32, 8, 64
NCORES = 8
HL = HQ // NCORES        # 4 local q heads
T = B * S
P = 128
EO = E // P              # 16 e-chunks
NCH = S // P             # 16 token chunks per batch
NKT = S // P             # 16 key chunks
NQB = S // 512           # 4 q blocks of 512

_CACHED = {}


def _build_nc(reps=1, phases=("qkv", "att", "oproj")):
    phases = set(phases)
    import concourse.mybir as mybir
    import concourse.tile as tile
    from concourse import bacc
    from concourse import library_config
    from concourse.bass import ts
    from concourse.masks import make_identity

    f32 = mybir.dt.float32
    f32r = mybir.dt.float32r
    bf16 = mybir.dt.bfloat16
    Exp = mybir.ActivationFunctionType.Exp
    Copy = mybir.ActivationFunctionType.Copy

    nc = bacc.Bacc("TRN2", target_bir_lowering=False, debug=False)

    NG = reps * B * NCH  # total chunks emitted
    xt = nc.dram_tensor("xt", [P, B * NCH, EO, P], bf16, kind="ExternalInput").ap()
    wqkv = nc.dram_tensor("wqkv", [P, EO, 384], bf16, kind="ExternalInput").ap()
    wo = nc.dram_tensor("wo", [P, 2, E], bf16, kind="ExternalInput").ap()
    cosr = nc.dram_tensor("cosr", [P, NCH, 64], f32, kind="ExternalInput").ap()
    sinr = nc.dram_tensor("sinr", [P, NCH, 64], f32, kind="ExternalInput").ap()
    out = nc.dram_tensor("out", [T, E], bf16, kind="ExternalOutput").ap()

    pools = {}
    state = {}

    def alloc_qkv(bb):
        qp, kqp = pools["qp"], pools["kqp"]
        qcomb = qp.tile([P, 2, S], bf16, name=f"q_{bb}", tag="q")
        kq_t = kqp.tile([P, S], bf16, name=f"kq_{bb}", tag="kq")
        state[("q", bb)] = qcomb
        state[("kq", bb)] = kq_t
        state[("v", bb)] = []
        emit_x_dma(bb, 0)

    def emit_qkv_chunk(bb, sc):
        b = bb % B
        xp, mmps = pools["xp"], pools["mmps"]
        qrp, qcp, qsp = pools["qrp"], pools["qcp"], pools["qsp"]
        vp = pools["vp"]
        wqkv_sb, cos_sb, sin_sb = pools["wqkv_sb"], pools["cos_sb"], pools["sin_sb"]
        ident_bf = pools["ident_bf"]
        qcomb = state[("q", bb)]
        kq_t = state[("kq", bb)]
        v_tiles = state[("v", bb)]
        if True:
            xc = state.pop(("xc", bb, sc))
            bq = mmps.tile([P, 6, 64], f32, name=f"bq_{bb}_{sc}", tag="mm")
            for e in range(EO):
                nc.tensor.matmul(
                    bq[:], xc[:, e], wqkv_sb[:, e],
                    start=(e == 0), stop=(e == EO - 1),
                )
            # fused rope over 5 slots (4 q heads + k): identical per-slot
            # formula since cos_sb = [cos|cos], sin_sb = [-sin|sin]
            qc_t = qcp.tile([P, 5, 64], f32, name=f"qc_{bb}_{sc}", tag="qc")
            qs_t = qsp.tile([P, 5, 64], f32, name=f"qs_{bb}_{sc}", tag="qs")
            qr = qrp.tile([P, 5, 64], bf16, name=f"qr_{bb}_{sc}", tag="qr")
            cosv = cos_sb[:, sc].unsqueeze(1).broadcast_to([P, 5, 64])
            sinn = sin_sb[:, sc, 0:32].unsqueeze(1).broadcast_to([P, 5, 32])
            sinp = sin_sb[:, sc, 32:64].unsqueeze(1).broadcast_to([P, 5, 32])
            nc.vector.tensor_mul(qc_t[:], bq[:, 0:5, :], cosv)
            nc.vector.tensor_mul(qs_t[:, :, 0:32], bq[:, 0:5, 32:64], sinn)
            nc.vector.tensor_mul(qs_t[:, :, 32:64], bq[:, 0:5, 0:32], sinp)
            nc.vector.tensor_add(qr[:], qc_t[:], qs_t[:])
            v_t = vp.tile([P, 65], bf16, name=f"v_{bb}_{sc}", tag="v")
            nc.vector.tensor_copy(out=v_t[:, 0:64], in_=bq[:, 5, :])
            nc.vector.memset(v_t[:, 64:65], 1.0)
            v_tiles.append(v_t)
            # transposes: q pairs and k
            for pair in range(2):
                psT = mmps.tile([P, P], bf16, name=f"psT_{bb}_{sc}_{pair}", tag="mm")
                nc.tensor.matmul(
                    psT[:], qr[:, 2 * pair : 2 * pair + 2, :], ident_bf[:],
                    is_transpose=True,
                )
                nc.vector.tensor_copy(
                    out=qcomb[:, pair, ts(sc, P)], in_=psT[:]
                )
            psK = mmps.tile([64, P], bf16, name=f"psK_{bb}_{sc}", tag="mm")
            nc.tensor.matmul(psK[:], qr[:, 4, :], ident_bf[:], is_transpose=True)
            nc.vector.tensor_copy(out=kq_t[0:64, ts(sc, P)], in_=psK[:])
            nc.vector.tensor_copy(out=kq_t[64:128, ts(sc, P)], in_=psK[:])

    def emit_x_dma(bb, sc):
        b = bb % B
        xp = pools["xp"]
        xc = xp.tile([P, EO, P], bf16, name=f"xc_{bb}_{sc}", tag="xc")
        nc.sync.dma_start(xc[:], xt[:, b * NCH + sc])
        state[("xc", bb, sc)] = xc

    def qkv_units(bb):
        # each unit prefetches the NEXT chunk's x DMA, then computes the
        # current chunk (whose DMA was issued one unit earlier)
        for sc in range(NCH):
            yield ("qkv", bb, sc)

    def emit_oproj_unit(u):
        _, bb, tch, et = u
        b = bb % B
        mmps, osp, wo_sb = pools["mmps"], pools["osp"], pools["wo_sb"]
        oTs = state[("oT", bb)]
        pso = mmps.tile([P, 512], f32, name=f"pso_{bb}_{tch}_{et}", tag="mm")
        for j in range(2):
            nc.tensor.matmul(
                pso[:], oTs[j][:, ts(tch, P)], wo_sb[:, j, ts(et, 512)],
                start=(j == 0), stop=(j == 1),
            )
        if et == 0:
            state[("ost", bb, tch)] = osp.tile(
                [P, E], bf16, name=f"os_{bb}_{tch}", tag="os"
            )
        ost = state[("ost", bb, tch)]
        nc.vector.tensor_copy(out=ost[:, ts(et, 512)], in_=pso[:])
        if et == 3:
            del state[("ost", bb, tch)]
            nc.sync.dma_start(
                out[b * S + tch * P : b * S + (tch + 1) * P, :], ost[:]
            )

    def oproj_units(bb):
        for tch in range(NCH):
            for et in range(4):
                yield ("oproj", bb, tch, et)

    def emit_unit(u):
        if u[0] == "qkv":
            bb, sc = u[1], u[2]
            if sc + 1 < NCH:
                emit_x_dma(bb, sc + 1)
            emit_qkv_chunk(bb, sc)
        else:
            emit_oproj_unit(u)

    def emit_attention(bb, unit_queues, oproj_q):
        from concourse.bass import ts as _ts
        scps, pops, mmps = pools["scps"], pools["pops"], pools["mmps"]
        expsp, rzp, rzsp, otp = pools["expsp"], pools["rzp"], pools["rzsp"], pools["otp"]
        qcomb = state[("q", bb)]
        kq_t = state[("kq", bb)]
        v_tiles = state[("v", bb)]
        oTs = [
            otp.tile([P, S], bf16, name=f"oT_{bb}_{pair}", tag="oT")
            for pair in range(2)
        ]
        state[("oT", bb)] = oTs
        # own-batch oproj units become available one q-block behind the
        # attention sweep (qtb-outer order: both pairs of qtb j finish
        # before qtb j+1 starts) and are drained via the kt interleave;
        # the last q-block's units carry over into the next batch
        for qtb in range(NQB):
            for pair in range(2):
                oT_t = oTs[pair]
                po0 = pops.tile([65, 512], f32, name=f"po0_{bb}_{pair}_{qtb}", tag="po")
                po1 = pops.tile([65, 512], f32, name=f"po1_{bb}_{pair}_{qtb}", tag="po")
                ex_tiles = {}
                # AV lags scores by one key-chunk so exp(kt) is done when
                # the in-order PE stream reaches av(kt)
                for kt in range(NKT + 1):
                    if kt < NKT:
                        sp = scps.tile(
                            [P, 1024], f32, name=f"sp_{bb}_{pair}_{qtb}_{kt}", tag="sp"
                        )
                        nc.tensor.matmul(
                            sp[:, 0:512], kq_t[0:64, _ts(kt, P)],
                            qcomb[0:64, pair, _ts(qtb, 512)], start=True, stop=True,
                        )
                        nc.tensor.matmul(
                            sp[:, 512:1024], kq_t[64:128, _ts(kt, P)],
                            qcomb[64:128, pair, _ts(qtb, 512)], start=True, stop=True,
                        )
                        ex = expsp.tile(
                            [P, 1024], bf16, name=f"ex_{bb}_{pair}_{qtb}_{kt}", tag="ex"
                        )
                        nc.scalar.activation(ex[:], sp[:], Exp)
                        ex_tiles[kt] = ex
                    if kt >= 1:
                        ak = kt - 1
                        ex = ex_tiles.pop(ak)
                        nc.tensor.matmul(
                            po0[:], v_tiles[ak][:], ex[:, 0:512],
                            start=(ak == 0), stop=(ak == NKT - 1),
                        )
                        nc.tensor.matmul(
                            po1[:], v_tiles[ak][:], ex[:, 512:1024],
                            start=(ak == 0), stop=(ak == NKT - 1),
                        )
                    # interleave own-batch out-proj and qkv (own chase for
                    # batch 0, next batch otherwise) units to keep PE dense
                    # while act paces the kt loop
                    if kt % 2 == 1 and oproj_q:
                        emit_unit(oproj_q.popleft())
                    qkv_cad, qkv_ph = (2, 0) if bb == 0 else (8, 4)
                    if kt % qkv_cad == qkv_ph and unit_queues:
                        qq = unit_queues.get("qkv")
                        if qq is not None:
                            u = next(qq, None)
                            if u is not None:
                                emit_unit(u)
                # drain po to SBUF with two fast DVE copies so the PSUM
                # banks recycle in ~1us; the slow reciprocal -> broadcast
                # -> normalize chain then runs on the SBUF copies while
                # the next q-block's AV matmuls proceed
                drained = []
                pbp = pools["pbp"]
                for hh, po in ((0, po0), (1, po1)):
                    zrow = rzp.tile([1, 512], f32, name=f"zr_{bb}_{pair}_{qtb}_{hh}", tag="rz")
                    nc.vector.tensor_copy(out=zrow[:], in_=po[64:65, :])
                    body = pbp.tile([64, 512], f32, name=f"pb_{bb}_{pair}_{qtb}_{hh}", tag="pb")
                    nc.vector.tensor_copy(out=body[:], in_=po[0:64, :])
                    drained.append((zrow, body))
                for hh, (zrow, body) in enumerate(drained):
                    rz = rzp.tile([1, 512], f32, name=f"rz_{bb}_{pair}_{qtb}_{hh}", tag="rz")
                    with nc.allow_low_precision(reason="recip of softmax denom"):
                        nc.vector.reciprocal(rz[:], zrow[:])
                    rzs = rzsp.tile([64, 512], f32, name=f"rzs_{bb}_{pair}_{qtb}_{hh}", tag="rzs")
                    nc.gpsimd.partition_broadcast(rzs[:], rz[:], channels=64)
                    nc.vector.tensor_mul(
                        oT_t[hh * 64 : (hh + 1) * 64, qtb * 512 : (qtb + 1) * 512],
                        body[:], rzs[:],
                    )
            # both pairs of this q-block done -> its 4 token chunks can
            # be projected; consumed by the kt interleave of later qtbs
            for tch in range(4 * qtb, 4 * qtb + 4):
                for et in range(4):
                    oproj_q.append(("oproj", bb, tch, et))
        return oproj_q

    from contextlib import ExitStack

    with tile.TileContext(nc) as tc:
        with ExitStack() as stk:
            ep = stk.enter_context
            const = ep(tc.tile_pool(name="const", bufs=1))
            xp = ep(tc.tile_pool(name="xp", bufs=3))
            qrp = ep(tc.tile_pool(name="qrp", bufs=3))
            qcp = ep(tc.tile_pool(name="qcp", bufs=2))
            qsp = ep(tc.tile_pool(name="qsp", bufs=2))
            qp = ep(tc.tile_pool(name="qp", bufs=2))
            kqp = ep(tc.tile_pool(name="kqp", bufs=2))
            vp = ep(tc.tile_pool(name="vp", bufs=34))
            expsp = ep(tc.tile_pool(name="expsp", bufs=4))
            otp = ep(tc.tile_pool(name="otp", bufs=4))
            rzp = ep(tc.tile_pool(name="rzp", bufs=6))
            rzsp = ep(tc.tile_pool(name="rzsp", bufs=2))
            pbp = ep(tc.tile_pool(name="pbp", bufs=4))
            osp = ep(tc.tile_pool(name="osp", bufs=2))
            scps = ep(tc.tile_pool(name="scps", bufs=2, space="PSUM"))
            pops = ep(tc.tile_pool(name="pops", bufs=2, space="PSUM"))
            mmps = ep(tc.tile_pool(name="mmps", bufs=2, space="PSUM"))
            pools.update(
                xp=xp, qrp=qrp, qcp=qcp, qsp=qsp,
                qp=qp, kqp=kqp, vp=vp, expsp=expsp, otp=otp, rzp=rzp,
                rzsp=rzsp, pbp=pbp, osp=osp, scps=scps, pops=pops, mmps=mmps,
            )
            nc.gpsimd.load_library(library_config.attn)
            ident = const.tile([P, P], f32)
            make_identity(nc, ident)
            ident_bf = const.tile([P, P], bf16)
            nc.vector.tensor_copy(out=ident_bf[:], in_=ident[:])
            wqkv_sb = const.tile([P, EO, 384], bf16)
            nc.scalar.dma_start(wqkv_sb[:], wqkv)
            wo_sb = const.tile([P, 2, E], bf16)
            nc.scalar.dma_start(wo_sb[:], wo)
            cos_sb = const.tile([P, NCH, 64], f32)
            nc.scalar.dma_start(cos_sb[:], cosr)
            sin_sb = const.tile([P, NCH, 64], f32)
            nc.scalar.dma_start(sin_sb[:], sinr)
            pools.update(
                ident_bf=ident_bf, wqkv_sb=wqkv_sb, wo_sb=wo_sb,
                cos_sb=cos_sb, sin_sb=sin_sb,
            )

            import itertools
            from collections import deque

            NBB = reps * B
            CHASE = NCH // 2  # batch-0 chunks emitted up front; rest chase
            oproj_q = deque()
            if "qkv" in phases:
                alloc_qkv(0)
                pend0 = list(qkv_units(0))
                for u in pend0[:CHASE]:
                    emit_unit(u)
                pend0 = pend0[CHASE:]
            for bb in range(NBB):
                queues = {}
                if "qkv" in phases:
                    chase = pend0 if bb == 0 else []
                    if bb == 0 and "att" not in phases:
                        for u in pend0:
                            emit_unit(u)
                        chase = []
                    if bb + 1 < NBB:
                        alloc_qkv(bb + 1)
                        queues["qkv"] = itertools.chain(
                            chase, qkv_units(bb + 1)
                        )
                    elif chase:
                        queues["qkv"] = iter(chase)
                if "att" in phases:
                    oproj_q = emit_attention(bb, queues, oproj_q)
                elif "qkv" in phases and bb + 1 < NBB:
                    for u in queues["qkv"]:
                        emit_unit(u)
                # drop stale state to keep dict small
                for key in [k for k in state if isinstance(k[1], int) and k[1] < bb - 1]:
                    del state[key]
            while oproj_q:
                emit_unit(oproj_q.popleft())

    nc.compile()
    return nc


def _prep_in_maps(inputs):
    import ml_dtypes

    bf = ml_dtypes.bfloat16
    x = np.asarray(inputs["x"], dtype=np.float32)
    cos = np.asarray(inputs["rope_cos"], dtype=np.float32)[0, :, 0, :]  # [S, 32]
    sin = np.asarray(inputs["rope_sin"], dtype=np.float32)[0, :, 0, :]
    Wq = np.asarray(inputs["Wq"], dtype=np.float32)
    Wk = np.asarray(inputs["Wk"], dtype=np.float32)
    Wv = np.asarray(inputs["Wv"], dtype=np.float32)
    Wo = np.asarray(inputs["Wo"], dtype=np.float32)

    # x chunks: [pe, g, eo, tok]
    xr = x.reshape(T // P, P, EO, P).transpose(3, 0, 2, 1)
    xr = np.ascontiguousarray(xr).astype(bf)

    # cos/sin tiles: [tok_p, sc, 64]
    cos_t = cos.reshape(NCH, P, 32).transpose(1, 0, 2)  # [P, NCH, 32]
    cos_sb = np.concatenate([cos_t, cos_t], axis=2)  # [P, NCH, 64]
    sin_t = sin.reshape(NCH, P, 32).transpose(1, 0, 2)
    sin_sb = np.concatenate([-sin_t, sin_t], axis=2)
    cos_sb = np.ascontiguousarray(cos_sb)
    sin_sb = np.ascontiguousarray(sin_sb)

    in_maps = []
    for c in range(NCORES):
        wq_c = Wq[:, HL * c : HL * (c + 1), :].reshape(E, HL * D)
        wfull = np.concatenate([wq_c, Wk[:, c, :], Wv[:, c, :]], axis=1)  # [E,384]
        wqkv_c = np.ascontiguousarray(
            wfull.reshape(EO, P, 384).transpose(1, 0, 2)
        ).astype(bf)
        wo_c = np.ascontiguousarray(
            Wo[HL * c : HL * (c + 1)].reshape(2, P, E).transpose(1, 0, 2)
        ).astype(bf)
        in_maps.append(
            {
                "xt": xr,
                "wqkv": wqkv_c,
                "wo": wo_c,
                "cosr": cos_sb,
                "sinr": sin_sb,
            }
        )
    return in_maps


def kernel(**inputs):
    from concourse.bass_utils import run_bass_kernel_spmd

    if "nc" not in _CACHED:
        _CACHED["nc"] = _build_nc()
    nc = _CACHED["nc"]

    in_maps = _prep_in_maps(inputs)
    trace = bool(int(os.environ.get("ATTN_TRACE", "0")))
    res = run_bass_kernel_spmd(
        nc, in_maps, core_ids=list(range(NCORES)), trace=trace
    )
    _CACHED["last_results"] = res

    acc = res.results[0]["out"].astype(np.float32)
    for c in range(1, NCORES):
        acc = acc + res.results[c]["out"].astype(np.float32)
    return np.ascontiguousarray(acc.reshape(B, S, E))

